# revision 6
# baseline (speedup 1.0000x reference)
"""CrossAttentionBlock Trainium2 kernel (8 NeuronCores, data-parallel over batch).

Problem: B=8 batch of channel-attention blocks.
  q/k/v = 1x1-conv projections (512->512) over L=64*64=4096 tokens,
  8 heads of d=64 channels, attention over CHANNELS (d x d logits,
  contracted over all 4096 tokens), softmax over the second channel
  axis, AV over channels, then a 1x1-conv output projection.

Sharding: batch b -> core b (8 cores). Each core runs the identical
program on its own batch element; weights are broadcast.

Key algebraic fold: with An = blockdiag(diag(r) exp(S*scale)) the whole
tail   out = Wo @ An @ (Wv @ Xv + bv 1^T) + bo 1^T
collapses to
       out = W3 @ Xv + b3 1^T,   W3 = (Wo An) Wv,  b3 = (Wo An) bv + bo
so the v projection (a quarter of all GEMM work) becomes a one-time
512x512x512 fold on-device, and phase B streams the RAW fp16 input.

Per-core dataflow (all matmul operands fp16, PSUM f32):
  phase A (streamed over 8 chunks of 512 tokens):
    xq/xk chunk DMA -> SBUF [128, 4, 512] (channel-major)
    qT/kT = x.T @ wqT + bq   -> [128 tok, 512 ch] per 128-token tile
    logits S[p] += qT_pair.T @ kT_pair accumulated in one PSUM bank
      (pairs of heads packed into 128x128 blocks; diagonal 64-blocks valid)
  softmax over the free axis of the valid 64x64 blocks (exp via ACT,
    rowsum, reciprocal), then the folds:
      W2T = blockdiag(An).T-free matmuls with woT chunks
      W3T = Wv-chunk.T @ W2T   (16 matmuls), b3 = W2.T-free @ bv + bo
  phase B (streamed over 8 chunks of 512 tokens):
    out[m] = W3T_chunks.T @ xv_chunk + b3 -> DMA out (fp16, channel-major)
"""

import os
import sys

for _p in ("/opt/trn_rl_repo", "/root/.axon_site/_ro/trn_rl_repo"):
    if os.path.isdir(_p):
        if _p not in sys.path:
            sys.path.insert(0, _p)
        break

import numpy as np

import concourse.bass as bass  # noqa: F401  (import keeps bass registered)
import concourse.mybir as mybir
import concourse.tile as tile
from concourse import bacc
from concourse.bass_utils import run_bass_kernel_spmd

F32 = mybir.dt.float32
F32R = mybir.dt.float32r
FP16 = mybir.dt.float16

B = 8
C = 512
L = 4096
NH = 8
D = 64
P = 128
CC = C // P  # 4 channel chunks of 128
NPAIR = NH // 2  # 4 head pairs -> 128-channel chunks
LCHUNK = 512
NLC = L // LCHUNK  # 8 token chunks
NLT = LCHUNK // P  # 4 token tiles of 128 per chunk
SCALE = 1.0 / float(np.sqrt(L))

AF = mybir.ActivationFunctionType
AX = mybir.AxisListType


def build_nc(n_iter=1, xin_bufs=2, qkp_bufs=3, psqk_bufs=3, outp_bufs=3, pso_bufs=2):
    nc = bacc.Bacc()

    xq = nc.declare_dram_parameter("xq", [C, L], FP16, isOutput=False)
    xk = nc.declare_dram_parameter("xk", [C, L], FP16, isOutput=False)
    xv = nc.declare_dram_parameter("xv", [C, L], FP16, isOutput=False)
    wqT = nc.declare_dram_parameter("wqT", [C, C], FP16, isOutput=False)
    wkT = nc.declare_dram_parameter("wkT", [C, C], FP16, isOutput=False)
    wv = nc.declare_dram_parameter("wv", [C, C], FP16, isOutput=False)
    woT = nc.declare_dram_parameter("woT", [C, C], FP16, isOutput=False)
    bq = nc.declare_dram_parameter("bq", [C], F32, isOutput=False)
    bk = nc.declare_dram_parameter("bk", [C], F32, isOutput=False)
    bv_pm = nc.declare_dram_parameter("bv_pm", [P, CC], F32, isOutput=False)
    bo_pm = nc.declare_dram_parameter("bo_pm", [P, CC], F32, isOutput=False)
    out = nc.declare_dram_parameter("out", [C, L], FP16, isOutput=True)

    # channel-chunked views: c = cc*128 + p
    xq_v = xq.rearrange("(cc p) l -> p cc l", p=P)
    xk_v = xk.rearrange("(cc p) l -> p cc l", p=P)
    xv_v = xv.rearrange("(cc p) l -> p cc l", p=P)
    out_v = out.rearrange("(m p) l -> p m l", p=P)
    wq_view = wqT.rearrange("(cc p) o -> p cc o", p=P)
    wk_view = wkT.rearrange("(cc p) o -> p cc o", p=P)
    wv_view = wv.rearrange("(ee p) c -> p ee c", p=P)
    wo_view = woT.rearrange("(cc p) o -> p cc o", p=P)

    with tile.TileContext(nc) as tc:
        with tc.tile_pool(name="const", bufs=1) as const:
            bq_sb = const.tile([P, C], F32)
            bk_sb = const.tile([P, C], F32)
            bv_sb = const.tile([P, CC], F32)
            bo_sb = const.tile([P, CC], F32)
            b3_sb = const.tile([P, CC], F32)
            # persistent weights; wq/wk pre-transposed on host to [c_in, c_out],
            # wv in natural [c_out, c_in] layout (fold lhsT), wo transposed
            wq_sb = const.tile([P, CC, C], FP16)
            wk_sb = const.tile([P, CC, C], FP16)
            wv_sb = const.tile([P, CC, C], FP16)
            wo_sb = const.tile([P, CC, C], FP16)
            w2_sb = const.tile([P, NPAIR, C], FP16)  # W2T = (blockdiag(An) @ woT)
            w3_sb = const.tile([P, CC, C], FP16)  # W3T = WvT @ W2T
            r_all = const.tile([P, NPAIR], F32)  # 1/rowsum per d-channel

            def one_pass(it):
                # ---------------- phase A: q/k projections + logits ----------
                with tc.tile_pool(name=f"psS{it}", bufs=1, space="PSUM") as psS_pool:
                    psum_S = psS_pool.tile([P, 512], F32)
                    with (
                        tc.tile_pool(name=f"xin{it}", bufs=xin_bufs) as xin,
                        tc.tile_pool(name=f"qkp{it}", bufs=qkp_bufs) as qkp,
                        tc.tile_pool(name=f"psqk{it}", bufs=psqk_bufs, space="PSUM") as psqk,
                    ):
                        # logits are emitted one l_tile behind the projections
                        # so the in-order PE never waits on the DVE bias-add
                        pend = []

                        def emit_logits(qT, kT, lt):
                            for pp in range(NPAIR):
                                co = pp * P
                                nc.tensor.matmul(
                                    psum_S[:, co : co + P],
                                    qT[:, co : co + P],
                                    kT[:, co : co + P],
                                    # start clears has_written BANK-wide: only
                                    # the first matmul touching the bank sets it
                                    start=(lt == 0 and pp == 0),
                                    stop=(lt == NLC * NLT - 1 and pp == NPAIR - 1),
                                )

                        for lc in range(NLC):
                            ls = lc * LCHUNK
                            xq_t = xin.tile([P, CC, LCHUNK], FP16, tag="xq_t")
                            xk_t = xin.tile([P, CC, LCHUNK], FP16, tag="xk_t")
                            if lc == 0:
                                # startup: interleave per channel-chunk so each
                                # accumulation step's operands arrive just in
                                # time (DMAs complete in issue order)
                                for cc in range(CC):
                                    nc.sync.dma_start(
                                        xq_t[:, cc, :], xq_v[:, cc, ls : ls + LCHUNK]
                                    )
                                    nc.sync.dma_start(wq_sb[:, cc, :], wq_view[:, cc, :])
                                for cc in range(CC):
                                    nc.sync.dma_start(
                                        xk_t[:, cc, :], xk_v[:, cc, ls : ls + LCHUNK]
                                    )
                                    nc.sync.dma_start(wk_sb[:, cc, :], wk_view[:, cc, :])
                                nc.sync.dma_start(bq_sb[:], bq[:].partition_broadcast(P))
                                nc.sync.dma_start(bk_sb[:], bk[:].partition_broadcast(P))
                                nc.sync.dma_start(bv_sb[:], bv_pm[:])
                                nc.sync.dma_start(bo_sb[:], bo_pm[:])
                            else:
                                if lc == 2:
                                    # wv/wo are first used at the fold, after
                                    # all 8 chunks of phase A
                                    for cc in range(CC):
                                        nc.sync.dma_start(
                                            wv_sb[:, cc, :], wv_view[:, cc, :]
                                        )
                                        nc.sync.dma_start(
                                            wo_sb[:, cc, :], wo_view[:, cc, :]
                                        )
                                for cc in range(CC):
                                    nc.sync.dma_start(
                                        xq_t[:, cc, :], xq_v[:, cc, ls : ls + LCHUNK]
                                    )
                                    nc.sync.dma_start(
                                        xk_t[:, cc, :], xk_v[:, cc, ls : ls + LCHUNK]
                                    )

                            for ltl in range(NLT):
                                lt = lc * NLT + ltl
                                to = ltl * P
                                ps_q = psqk.tile([P, C], F32, tag="ps_q")
                                ps_k = psqk.tile([P, C], F32, tag="ps_k")
                                for cc in range(CC):
                                    nc.tensor.matmul(
                                        ps_q[:],
                                        xq_t[:, cc, to : to + P],
                                        wq_sb[:, cc, :],
                                        start=(cc == 0),
                                        stop=(cc == CC - 1),
                                    )
                                qT = qkp.tile([P, C], FP16, tag="qT")
                                nc.vector.tensor_add(qT[:], ps_q[:], bq_sb[:])
                                for cc in range(CC):
                                    nc.tensor.matmul(
                                        ps_k[:],
                                        xk_t[:, cc, to : to + P],
                                        wk_sb[:, cc, :],
                                        start=(cc == 0),
                                        stop=(cc == CC - 1),
                                    )
                                kT = qkp.tile([P, C], FP16, tag="kT")
                                nc.vector.tensor_add(kT[:], ps_k[:], bk_sb[:])

                                if pend:
                                    emit_logits(*pend.pop())
                                pend.append((qT, kT, lt))

                        if pend:
                            emit_logits(*pend.pop())

                    # ---------------- softmax + folds ----------------
                    # vectorized over all 4 pairs; no max-subtraction
                    # (|S*scale| is O(1) here, exp is safe in fp32)
                    with (
                        tc.tile_pool(name=f"smx{it}", bufs=1) as smx,
                        tc.tile_pool(name=f"pstr{it}", bufs=2, space="PSUM") as pstr,
                    ):
                        S_v = psum_S[:].rearrange("p (pp q) -> p pp q", q=P)
                        attn_all = smx.tile([P, NPAIR, P], F32, tag="attn_all")
                        nc.vector.memset(attn_all[:], 0.0)
                        z_all = smx.tile([P, NPAIR], F32, tag="z_all")
                        nc.scalar.activation(
                            attn_all[0:D, :, 0:D],
                            S_v[0:D, :, 0:D],
                            AF.Exp,
                            bias=0.0,
                            scale=SCALE,
                        )
                        nc.scalar.activation(
                            attn_all[D:P, :, D:P],
                            S_v[D:P, :, D:P],
                            AF.Exp,
                            bias=0.0,
                            scale=SCALE,
                        )
                        nc.vector.reduce_sum(
                            z_all[0:D, :], attn_all[0:D, :, 0:D], axis=AX.X
                        )
                        nc.vector.reduce_sum(
                            z_all[D:P, :], attn_all[D:P, :, D:P], axis=AX.X
                        )
                        nc.vector.reciprocal(r_all[:], z_all[:])
                        attn_n = smx.tile([P, NPAIR, P], FP16, tag="attn_n")
                        for pp in range(NPAIR):
                            nc.vector.tensor_scalar_mul(
                                attn_n[:, pp, :],
                                attn_all[:, pp, :],
                                r_all[:, pp : pp + 1],
                            )
                            ps_w = pstr.tile([P, C], F32, tag="ps_w")
                            nc.tensor.matmul(
                                ps_w[:],
                                attn_n[:, pp, :],
                                wo_sb[:, pp, :],
                                start=True,
                                stop=True,
                            )
                            nc.vector.tensor_copy(w2_sb[:, pp, :], ps_w[:])
                        # W3T[c,:] = sum_e Wv[e,c] * W2T[e,:]
                        for cc in range(CC):
                            ps_w3 = pstr.tile([P, C], F32, tag="ps_w3")
                            for ee in range(CC):
                                nc.tensor.matmul(
                                    ps_w3[:],
                                    wv_sb[:, ee, cc * P : (cc + 1) * P],
                                    w2_sb[:, ee, :],
                                    start=(ee == 0),
                                    stop=(ee == CC - 1),
                                )
                            nc.vector.tensor_copy(w3_sb[:, cc, :], ps_w3[:])
                        # b3[o] = sum_e W2T[e,o] bv[e] + bo[o], o partition-major
                        bv16 = smx.tile([P, NPAIR], FP16, tag="bv16")
                        nc.vector.tensor_copy(bv16[:], bv_sb[:])
                        ps_b3 = pstr.tile([P, CC], F32, tag="ps_b3")
                        for m in range(CC):
                            for pp in range(NPAIR):
                                nc.tensor.matmul(
                                    ps_b3[:, m : m + 1],
                                    w2_sb[:, pp, m * P : (m + 1) * P],
                                    bv16[:, pp : pp + 1],
                                    start=(pp == 0),
                                    stop=(pp == NPAIR - 1),
                                )
                        nc.vector.tensor_add(b3_sb[:], ps_b3[:], bo_sb[:])

                # ---------------- phase B: out = W3 @ xv + b3 ----------------
                with (
                    tc.tile_pool(name=f"xvin{it}", bufs=xin_bufs) as xvin,
                    tc.tile_pool(name=f"outp{it}", bufs=outp_bufs) as outp,
                    tc.tile_pool(name=f"pso{it}", bufs=pso_bufs, space="PSUM") as pso,
                ):
                    for lc in range(NLC):
                        ls = lc * LCHUNK
                        xv_t = xvin.tile([P, CC, LCHUNK], FP16, tag="xv_t")
                        for cc in range(CC):
                            nc.sync.dma_start(
                                xv_t[:, cc, :], xv_v[:, cc, ls : ls + LCHUNK]
                            )
                        for m in range(CC):
                            ps_o = pso.tile([P, LCHUNK], F32, tag="ps_o")
                            for cc in range(CC):
                                nc.tensor.matmul(
                                    ps_o[:],
                                    w3_sb[:, cc, m * P : (m + 1) * P],
                                    xv_t[:, cc, :],
                                    start=(cc == 0),
                                    stop=(cc == CC - 1),
                                )
                            o_t = outp.tile([P, LCHUNK], FP16, tag="o_t")
                            nc.scalar.activation(
                                o_t[:],
                                ps_o[:],
                                AF.Identity,
                                bias=b3_sb[:, m : m + 1],
                                scale=1.0,
                            )
                            nc.sync.dma_start(out_v[:, m, ls : ls + LCHUNK], o_t[:])

            for it in range(n_iter):
                one_pass(it)

    nc.compile()
    return nc


_NC_CACHE = None


def _get_nc():
    global _NC_CACHE
    if _NC_CACHE is None:
        _NC_CACHE = build_nc()
    return _NC_CACHE


def _prep_in_maps(query, key, value, wq, bq, wk, bk, wv, bv, wo, bo):
    f16 = np.float16

    def f32(a):
        return np.ascontiguousarray(np.asarray(a, dtype=np.float32))

    query = np.ascontiguousarray(np.asarray(query, np.float32).astype(f16))
    key = np.ascontiguousarray(np.asarray(key, np.float32).astype(f16))
    value = np.ascontiguousarray(np.asarray(value, np.float32).astype(f16))
    shared = {
        "wqT": np.ascontiguousarray(np.asarray(wq, np.float32).T.astype(f16)),
        "wkT": np.ascontiguousarray(np.asarray(wk, np.float32).T.astype(f16)),
        "wv": np.ascontiguousarray(np.asarray(wv, np.float32).astype(f16)),
        "woT": np.ascontiguousarray(np.asarray(wo, np.float32).T.astype(f16)),
        "bq": f32(bq),
        "bk": f32(bk),
        "bv_pm": f32(np.asarray(bv, np.float32).reshape(CC, P).T),
        "bo_pm": f32(np.asarray(bo, np.float32).reshape(CC, P).T),
    }
    in_maps = []
    for b in range(B):
        in_maps.append(
            {
                "xq": query[b].reshape(C, L),
                "xk": key[b].reshape(C, L),
                "xv": value[b].reshape(C, L),
                **shared,
            }
        )
    return in_maps


def kernel(query, key, value, wq, bq, wk, bk, wv, bv, wo, bo):
    nc = _get_nc()
    in_maps = _prep_in_maps(query, key, value, wq, bq, wk, bk, wv, bv, wo, bo)
    res = run_bass_kernel_spmd(nc, in_maps, core_ids=list(range(B)))
    out = np.stack([res.results[b]["out"] for b in range(B)], axis=0)
    return out.reshape(B, C, 64, 64).astype(np.float32)


if __name__ == "__main__":
    rng = np.random.default_rng(0)
    sh = dict(
        query=rng.standard_normal((B, C, 64, 64), dtype=np.float32),
        key=rng.standard_normal((B, C, 64, 64), dtype=np.float32),
        value=rng.standard_normal((B, C, 64, 64), dtype=np.float32),
        wq=rng.standard_normal((C, C), dtype=np.float32) / np.sqrt(C),
        bq=rng.standard_normal((C,), dtype=np.float32) / np.sqrt(C),
        wk=rng.standard_normal((C, C), dtype=np.float32) / np.sqrt(C),
        bk=rng.standard_normal((C,), dtype=np.float32) / np.sqrt(C),
        wv=rng.standard_normal((C, C), dtype=np.float32) / np.sqrt(C),
        bv=rng.standard_normal((C,), dtype=np.float32) / np.sqrt(C),
        wo=rng.standard_normal((C, C), dtype=np.float32) / np.sqrt(C),
        bo=rng.standard_normal((C,), dtype=np.float32) / np.sqrt(C),
    )
    o = kernel(**sh)
    print("kernel output:", o.shape, o.dtype, float(np.abs(o).max()))


# revision 10
# speedup vs baseline: 1.3412x; 1.3412x over previous
"""CrossAttentionBlock Trainium2 kernel (8 NeuronCores, data-parallel over batch).

Problem: B=8 batch of channel-attention blocks.
  q/k/v = 1x1-conv projections (512->512) over L=64*64=4096 tokens,
  8 heads of d=64 channels, attention over CHANNELS (d x d logits,
  contracted over all 4096 tokens), softmax over the second channel
  axis, AV over channels, then a 1x1-conv output projection.

Sharding: batch b -> core b (8 cores). Each core runs the identical
program on its own batch element; weights are broadcast.

Two algebraic folds turn the whole block into three 512-sized GEMMs:

1. Gram-matrix logits. The logits contract q/k over all 4096 tokens:
     S = (Wq Xq + bq 1^T)(Wk Xk + bk 1^T)^T
       = Wq (Xq Xk^T) Wk^T + aq bk^T + bq ck^T
   with host-precomputed rank-1 rows aq = Wq (Xq 1) + L bq, ck = Wk (Xk 1).
   The device computes H = Xk Xq^T (one 512x512xL GEMM from XBAR-transposed
   token-major tiles), then U = H^T-free @ WkT, then the block-diagonal
   S = WqT^T-free @ U + two tiny outer products. This replaces the full
   q,k projections + logits (2.4 GMAC) with 1.2 GMAC and removes the
   per-token bias adds entirely.

2. v-projection fold. With An = blockdiag(diag(r) exp(S*scale)):
     out = Wo An (Wv Xv + bv 1^T) + bo 1^T = W3 Xv + b3 1^T
     W3 = (Wo An) Wv,  b3 = (Wo An) bv + bo
   so phase B streams the RAW fp16 input through one GEMM.

Scheduling notes (what made it fast, in order of discovery):
- DMA-issue sequencer cost is ~0.6 us per DMA instruction; everything is
  batched (one XBAR-transpose per 512-token chunk, one DMA per weight,
  one output DMA per chunk) and split across the SP queue (inputs) and
  ACT queue (xv + outputs) so no sequencer serializes.
- All working buffers (SBUF and PSUM) are persistent allocations with
  manual rotation rather than per-iteration pools, so back-to-back
  kernel executions overlap: the next run's input transposes prefetch
  during the current run's phase B.
- PSUM bank map: 0-3 G/U accumulation then W3 fold; 4 S logits, later
  reused as third phase-B accumulator; 5 W2 fold then b3; 6-7 phase B.
- PE p-states (0.65/1.2/2.4 GHz ramp) punish idle gaps: the PE stream
  is kept continuous so matmuls run at the hot clock.

All matmul operands fp16 (PSUM f32). Per-core per-run: PE ~152K cycles,
HBM ~19 MB.
"""

import os
import sys

for _p in ("/opt/trn_rl_repo", "/root/.axon_site/_ro/trn_rl_repo"):
    if os.path.isdir(_p):
        if _p not in sys.path:
            sys.path.insert(0, _p)
        break

import numpy as np

import concourse.bass as bass  # noqa: F401  (import keeps bass registered)
import concourse.mybir as mybir
import concourse.tile as tile
from concourse import bacc
from concourse.bass_utils import run_bass_kernel_spmd

F32 = mybir.dt.float32
FP16 = mybir.dt.float16

B = 8
C = 512
L = 4096
NH = 8
D = 64
P = 128
CC = C // P  # 4 channel chunks of 128
NPAIR = NH // 2  # 4 head pairs -> 128-channel chunks
LCHUNK = 512
NLC = L // LCHUNK  # 8 token chunks
NSUB = LCHUNK // P  # 4 token-major sub-tiles per transposed chunk
SCALE = 1.0 / float(np.sqrt(L))
XTB = 3  # phase-A transposed-chunk buffers
XVB = 3  # phase-B input buffers
OTB = 2  # phase-B output buffers

AF = mybir.ActivationFunctionType
AX = mybir.AxisListType

# rows_sb indices
R_AQ, R_BQ, R_BK, R_CK = 0, 1, 2, 3


def build_nc(n_iter=1):
    nc = bacc.Bacc()

    xq = nc.declare_dram_parameter("xq", [C, L], FP16, isOutput=False)
    xk = nc.declare_dram_parameter("xk", [C, L], FP16, isOutput=False)
    xv = nc.declare_dram_parameter("xv", [C, L], FP16, isOutput=False)
    wqT = nc.declare_dram_parameter("wqT", [C, C], FP16, isOutput=False)
    wkT = nc.declare_dram_parameter("wkT", [C, C], FP16, isOutput=False)
    wv = nc.declare_dram_parameter("wv", [C, C], FP16, isOutput=False)
    woT = nc.declare_dram_parameter("woT", [C, C], FP16, isOutput=False)
    rows = nc.declare_dram_parameter("rows", [4, C], FP16, isOutput=False)
    bvo_pm = nc.declare_dram_parameter("bvo_pm", [P, 2 * CC], F32, isOutput=False)
    out = nc.declare_dram_parameter("out", [C, L], FP16, isOutput=True)

    xv_v = xv.rearrange("(cc p) l -> p cc l", p=P)
    out_v = out.rearrange("(m p) l -> p m l", p=P)
    wq_view = wqT.rearrange("(cc p) o -> p cc o", p=P)
    wk_view = wkT.rearrange("(cc p) o -> p cc o", p=P)
    wv_view = wv.rearrange("(ee p) c -> p ee c", p=P)
    wo_view = woT.rearrange("(cc p) o -> p cc o", p=P)

    with tile.TileContext(nc) as tc:
        with (
            tc.tile_pool(name="const", bufs=1) as const,
            tc.tile_pool(name="psum", bufs=1, space="PSUM") as psum,
        ):
            bvo_sb = const.tile([P, 2 * CC], F32)
            b3_sb = const.tile([P, CC], F32)
            bv16_sb = const.tile([P, CC], FP16)
            rows_sb = const.tile([1, 4, C], FP16)
            wq_sb = const.tile([P, CC, C], FP16)
            wk_sb = const.tile([P, CC, C], FP16)
            wv_sb = const.tile([P, CC, C], FP16)
            wo_sb = const.tile([P, CC, C], FP16)
            H_sb = const.tile([P, CC, C], FP16)  # H = Xk Xq^T chunks
            U_sb = const.tile([P, CC, C], FP16)  # U = H^T WkT chunks
            w2_sb = const.tile([P, NPAIR, C], FP16)  # W2T = blockdiag(An) @ woT
            w3_sb = const.tile([P, CC, C], FP16)  # W3T = WvT @ W2T
            r_all = const.tile([P, NPAIR], F32)
            attn_all = const.tile([P, NPAIR, P], F32)
            attn_n = const.tile([P, NPAIR, P], FP16)
            z_all = const.tile([P, NPAIR], F32)
            # rotated working buffers
            xq_b = const.tile([P, XTB, NSUB, C], FP16)
            xk_b = const.tile([P, XTB, NSUB, C], FP16)
            xv_b = const.tile([P, XVB, CC, LCHUNK], FP16)
            o_b = const.tile([P, OTB, CC, LCHUNK], FP16)

            # PSUM bank map (8 banks of 512 f32):
            psum_G = psum.tile([P, CC, 512], F32)  # banks 0-3: G/U, then W3
            psum_S = psum.tile([P, 512], F32)  # bank 4: S; later 3rd ps_o
            ps_wb = psum.tile([P, 512], F32)  # bank 5: W2 fold, then b3
            ps_oA = psum.tile([P, 512], F32)  # bank 6
            ps_oB = psum.tile([P, 512], F32)  # bank 7
            ps_o_rot = [ps_oA, ps_oB, psum_S]

            # off-diagonal 64-blocks of the pair-packed attention are zero
            # for the whole program; exp only ever rewrites the diagonals
            nc.vector.memset(attn_all[:], 0.0)

            def one_pass(it):
                # ------------- phase A: H = Xk Xq^T (token-major) --------
                for lc in range(NLC):
                    ls = lc * LCHUNK
                    bi = (it * NLC + lc) % XTB
                    # one XBAR transpose per chunk: [C, 512] ->
                    # [128 tok, 4 sub, C] token-major
                    nc.sync.dma_start_transpose(
                        xq_b[:, bi, :, :], xq[:, ls : ls + LCHUNK]
                    )
                    nc.sync.dma_start_transpose(
                        xk_b[:, bi, :, :], xk[:, ls : ls + LCHUNK]
                    )
                    if lc == 1:
                        nc.sync.dma_start(wk_sb[:], wk_view[:])
                    elif lc == 2:
                        nc.sync.dma_start(wq_sb[:], wq_view[:])
                    elif lc == NLC - 1:
                        # fold-stage weights ride behind the last transposes
                        nc.sync.dma_start(wv_sb[:], wv_view[:])
                        nc.sync.dma_start(wo_sb[:], wo_view[:])
                        nc.sync.dma_start(rows_sb[:], rows[:])
                        nc.sync.dma_start(bvo_sb[:], bvo_pm[:])
                    for s in range(NSUB):
                        for cc in range(CC):
                            nc.tensor.matmul(
                                psum_G[:, cc, :],
                                xk_b[:, bi, s, cc * P : (cc + 1) * P],
                                xq_b[:, bi, s, :],
                                start=(lc == 0 and s == 0),
                                stop=(lc == NLC - 1 and s == NSUB - 1),
                            )

                # xv prefetch rides the ACT queue; lands during the U/S/fold
                # tail while the DMA engines are otherwise idle
                def prefetch_xv(lc):
                    if lc < NLC:
                        bi = (it * NLC + lc) % XVB
                        nc.scalar.dma_start(
                            xv_b[:, bi, :, :],
                            xv_v[:, :, lc * LCHUNK : (lc + 1) * LCHUNK],
                        )

                prefetch_xv(0)
                prefetch_xv(1)

                # H chunks PSUM -> SBUF fp16 (DVE/ACT split halves latency)
                for cc in range(CC):
                    if cc % 2 == 0:
                        nc.vector.tensor_copy(H_sb[:, cc, :], psum_G[:, cc, :])
                    else:
                        nc.scalar.activation(
                            H_sb[:, cc, :], psum_G[:, cc, :], AF.Copy
                        )
                # U[c,:] = sum_e H[e,c-slice]^T WkT[e,:] — same banks (WAR
                # on the H copies, range-tracked)
                for cc in range(CC):
                    for ee in range(CC):
                        nc.tensor.matmul(
                            psum_G[:, cc, :],
                            H_sb[:, ee, cc * P : (cc + 1) * P],
                            wk_sb[:, ee, :],
                            start=(ee == 0),
                            stop=(ee == CC - 1),
                        )
                for cc in range(CC):
                    if cc % 2 == 0:
                        nc.vector.tensor_copy(U_sb[:, cc, :], psum_G[:, cc, :])
                    else:
                        nc.scalar.activation(
                            U_sb[:, cc, :], psum_G[:, cc, :], AF.Copy
                        )
                # S (block-diagonal by head pair) + rank-1 bias terms
                for pp in range(NPAIR):
                    sl = slice(pp * P, (pp + 1) * P)
                    for cc in range(CC):
                        nc.tensor.matmul(
                            psum_S[:, sl],
                            wq_sb[:, cc, sl],
                            U_sb[:, cc, sl],
                            start=(pp == 0 and cc == 0),
                            stop=False,
                        )
                    nc.tensor.matmul(
                        psum_S[:, sl],
                        rows_sb[0:1, R_AQ, sl],
                        rows_sb[0:1, R_BK, sl],
                        start=False,
                        stop=False,
                    )
                    nc.tensor.matmul(
                        psum_S[:, sl],
                        rows_sb[0:1, R_BQ, sl],
                        rows_sb[0:1, R_CK, sl],
                        start=False,
                        stop=(pp == NPAIR - 1),
                    )

                # ---------------- softmax + folds ----------------
                S_v = psum_S[:].rearrange("p (pp q) -> p pp q", q=P)
                nc.scalar.activation(
                    attn_all[0:D, :, 0:D], S_v[0:D, :, 0:D], AF.Exp, 0.0, SCALE
                )
                nc.scalar.activation(
                    attn_all[D:P, :, D:P], S_v[D:P, :, D:P], AF.Exp, 0.0, SCALE
                )
                nc.vector.reduce_sum(z_all[0:D, :], attn_all[0:D, :, 0:D], axis=AX.X)
                nc.vector.reduce_sum(z_all[D:P, :], attn_all[D:P, :, D:P], axis=AX.X)
                nc.vector.reciprocal(r_all[:], z_all[:])
                nc.vector.tensor_copy(bv16_sb[:], bvo_sb[:, 0:CC])
                for pp in range(NPAIR):
                    nc.vector.tensor_scalar_mul(
                        attn_n[:, pp, :], attn_all[:, pp, :], r_all[:, pp : pp + 1]
                    )
                    nc.tensor.matmul(
                        ps_wb[:],
                        attn_n[:, pp, :],
                        wo_sb[:, pp, :],
                        start=True,
                        stop=True,
                    )
                    nc.vector.tensor_copy(w2_sb[:, pp, :], ps_wb[:])
                # W3T[c,:] = sum_e Wv[e,c] * W2T[e,:]  (into the G banks)
                for cc in range(CC):
                    for ee in range(CC):
                        nc.tensor.matmul(
                            psum_G[:, cc, :],
                            wv_sb[:, ee, cc * P : (cc + 1) * P],
                            w2_sb[:, ee, :],
                            start=(ee == 0),
                            stop=(ee == CC - 1),
                        )
                for cc in range(CC):
                    if cc % 2 == 0:
                        nc.vector.tensor_copy(w3_sb[:, cc, :], psum_G[:, cc, :])
                    else:
                        nc.scalar.activation(
                            w3_sb[:, cc, :], psum_G[:, cc, :], AF.Copy
                        )
                # b3[o] = sum_e W2T[e,o] bv[e] + bo[o], o partition-major
                ps_b3 = ps_wb[:, 0:CC]
                for m in range(CC):
                    for pp in range(NPAIR):
                        nc.tensor.matmul(
                            ps_b3[:, m : m + 1],
                            w2_sb[:, pp, m * P : (m + 1) * P],
                            bv16_sb[:, pp : pp + 1],
                            start=(m == 0 and pp == 0),
                            stop=(m == CC - 1 and pp == NPAIR - 1),
                        )
                nc.vector.tensor_add(b3_sb[:], ps_b3, bvo_sb[:, CC : 2 * CC])

                # ------------- phase B: out = W3 @ xv + b3 ---------------
                for lc in range(NLC):
                    ls = lc * LCHUNK
                    prefetch_xv(lc + 2)
                    xv_t = xv_b[:, (it * NLC + lc) % XVB]
                    o_t4 = o_b[:, lc % OTB]
                    for m in range(CC):
                        ps_o = ps_o_rot[(lc * CC + m) % 3]
                        for cc in range(CC):
                            nc.tensor.matmul(
                                ps_o[:],
                                w3_sb[:, cc, m * P : (m + 1) * P],
                                xv_t[:, cc, :],
                                start=(cc == 0),
                                stop=(cc == CC - 1),
                            )
                        nc.vector.tensor_scalar_add(
                            o_t4[:, m, :], ps_o[:], b3_sb[:, m : m + 1]
                        )
                    nc.scalar.dma_start(out_v[:, :, ls : ls + LCHUNK], o_t4[:])

            for it in range(n_iter):
                one_pass(it)

    nc.compile()
    return nc


_NC_CACHE = None


def _get_nc():
    global _NC_CACHE
    if _NC_CACHE is None:
        _NC_CACHE = build_nc()
    return _NC_CACHE


def _prep_in_maps(query, key, value, wq, bq, wk, bk, wv, bv, wo, bo):
    f16 = np.float16

    def f32(a):
        return np.ascontiguousarray(np.asarray(a, dtype=np.float32))

    wq = np.asarray(wq, np.float32)
    wk = np.asarray(wk, np.float32)
    bq32 = np.asarray(bq, np.float32)
    bk32 = np.asarray(bk, np.float32)
    query = np.asarray(query, np.float32).reshape(B, C, L)
    key = np.asarray(key, np.float32).reshape(B, C, L)
    q16 = np.ascontiguousarray(query.astype(f16))
    k16 = np.ascontiguousarray(key.astype(f16))
    v16 = np.ascontiguousarray(np.asarray(value, np.float32).astype(f16).reshape(B, C, L))
    # host-side rank-1 bias rows: aq = Wq (Xq 1) + L bq, ck = Wk (Xk 1)
    # (use the fp16-rounded inputs so corrections match the device data)
    sq = q16.astype(np.float32).sum(axis=2)  # [B, C]
    sk = k16.astype(np.float32).sum(axis=2)
    aq = sq @ wq.T + L * bq32[None, :]  # [B, C]
    ck = sk @ wk.T  # [B, C]
    bvo = np.concatenate(
        [
            np.asarray(bv, np.float32).reshape(CC, P).T,
            np.asarray(bo, np.float32).reshape(CC, P).T,
        ],
        axis=1,
    )
    shared = {
        "wqT": np.ascontiguousarray(wq.T.astype(f16)),
        "wkT": np.ascontiguousarray(wk.T.astype(f16)),
        "wv": np.ascontiguousarray(np.asarray(wv, np.float32).astype(f16)),
        "woT": np.ascontiguousarray(np.asarray(wo, np.float32).T.astype(f16)),
        "bvo_pm": f32(bvo),
    }
    bq16 = bq32.astype(f16)
    in_maps = []
    for b in range(B):
        rows_b = np.stack(
            [aq[b].astype(f16), bq16, bk32.astype(f16), ck[b].astype(f16)], axis=0
        )
        in_maps.append(
            {
                "xq": q16[b],
                "xk": k16[b],
                "xv": v16[b],
                "rows": np.ascontiguousarray(rows_b),
                **shared,
            }
        )
    return in_maps


def kernel(query, key, value, wq, bq, wk, bk, wv, bv, wo, bo):
    nc = _get_nc()
    in_maps = _prep_in_maps(query, key, value, wq, bq, wk, bk, wv, bv, wo, bo)
    res = run_bass_kernel_spmd(nc, in_maps, core_ids=list(range(B)))
    out = np.stack([res.results[b]["out"] for b in range(B)], axis=0)
    return out.reshape(B, C, 64, 64).astype(np.float32)


if __name__ == "__main__":
    rng = np.random.default_rng(0)
    sh = dict(
        query=rng.standard_normal((B, C, 64, 64), dtype=np.float32),
        key=rng.standard_normal((B, C, 64, 64), dtype=np.float32),
        value=rng.standard_normal((B, C, 64, 64), dtype=np.float32),
        wq=rng.standard_normal((C, C), dtype=np.float32) / np.sqrt(C),
        bq=rng.standard_normal((C,), dtype=np.float32) / np.sqrt(C),
        wk=rng.standard_normal((C, C), dtype=np.float32) / np.sqrt(C),
        bk=rng.standard_normal((C,), dtype=np.float32) / np.sqrt(C),
        wv=rng.standard_normal((C, C), dtype=np.float32) / np.sqrt(C),
        bv=rng.standard_normal((C,), dtype=np.float32) / np.sqrt(C),
        wo=rng.standard_normal((C, C), dtype=np.float32) / np.sqrt(C),
        bo=rng.standard_normal((C,), dtype=np.float32) / np.sqrt(C),
    )
    o = kernel(**sh)
    print("kernel output:", o.shape, o.dtype, float(np.abs(o).max()))


# revision 17
# speedup vs baseline: 1.6437x; 1.2255x over previous
"""CrossAttentionBlock Trainium2 kernel (8 NeuronCores, data-parallel over batch).

Problem: B=8 batch of channel-attention blocks.
  q/k/v = 1x1-conv projections (512->512) over L=64*64=4096 tokens,
  8 heads of d=64 channels, attention over CHANNELS (d x d logits,
  contracted over all 4096 tokens), softmax over the second channel
  axis, AV over channels, then a 1x1-conv output projection.

Sharding: batch b -> core b (8 cores). Each core runs the identical
program on its own batch element; weights are broadcast.

Two algebraic folds turn the whole block into three 512-sized GEMMs:

1. Gram-matrix logits. The logits contract q/k over all 4096 tokens:
     S = (Wq Xq + bq 1^T)(Wk Xk + bk 1^T)^T
       = Wq (Xq Xk^T) Wk^T + aq bk^T + bq ck^T
   with host-precomputed rank-1 rows aq = Wq (Xq 1) + L bq, ck = Wk (Xk 1).
   The device computes H = Xk Xq^T (one 512x512xL GEMM from XBAR-transposed
   token-major tiles), then U = H^T-free @ WkT, then the block-diagonal
   S = WqT^T-free @ U + two tiny outer products. This replaces the full
   q,k projections + logits (2.4 GMAC) with 1.2 GMAC and removes the
   per-token bias adds entirely.

2. v-projection fold. With An = blockdiag(diag(r) exp(S*scale)):
     out = Wo An (Wv Xv + bv 1^T) + bo 1^T = W3 Xv + b3 1^T
     W3 = (Wo An) Wv,  b3 = (Wo An) bv + bo
   so phase B streams the RAW fp16 input through one GEMM.

Scheduling notes (what made it fast, in order of discovery):
- DMA-issue sequencer cost is ~0.6 us per DMA instruction; everything is
  batched (one XBAR-transpose per 512-token chunk, one DMA per weight,
  one output DMA per chunk) and split across the SP queue (inputs) and
  ACT queue (xv + outputs) so no sequencer serializes.
- All working buffers (SBUF and PSUM) are persistent allocations with
  manual rotation rather than per-iteration pools, so back-to-back
  kernel executions overlap: the next run's input transposes prefetch
  during the current run's phase B.
- PSUM bank map: 0-3 G/U accumulation then W3 fold; 4 S logits, later
  reused as third phase-B accumulator; 5 W2 fold then b3; 6-7 phase B.
- PE p-states (0.65/1.2/2.4 GHz ramp) punish idle gaps: the PE stream
  is kept continuous so matmuls run at the hot clock.

All matmul operands fp16 (PSUM f32). Per-core per-run: PE ~152K cycles,
HBM ~19 MB.
"""

import os
import sys

for _p in ("/opt/trn_rl_repo", "/root/.axon_site/_ro/trn_rl_repo"):
    if os.path.isdir(_p):
        if _p not in sys.path:
            sys.path.insert(0, _p)
        break

import numpy as np

import concourse.bass as bass  # noqa: F401  (import keeps bass registered)
import concourse.mybir as mybir
import concourse.tile as tile
from concourse import bacc
from concourse.bass_utils import run_bass_kernel_spmd

F32 = mybir.dt.float32
FP16 = mybir.dt.float16

B = 8
C = 512
L = 4096
NH = 8
D = 64
P = 128
CC = C // P  # 4 channel chunks of 128
NPAIR = NH // 2  # 4 head pairs -> 128-channel chunks
LCHUNK = 512
NLC = L // LCHUNK  # 8 token chunks
NSUB = LCHUNK // P  # 4 token-major sub-tiles per transposed chunk
SCALE = 1.0 / float(np.sqrt(L))
XTB = 3  # phase-A transposed-chunk buffers
XVB = 3  # phase-B input buffers
OTB = 2  # phase-B output buffers

AF = mybir.ActivationFunctionType
AX = mybir.AxisListType

# rows_sb indices
R_AQ, R_BQ, R_BK, R_CK = 0, 1, 2, 3


def build_nc(n_iter=1):
    nc = bacc.Bacc()

    xq = nc.declare_dram_parameter("xq", [C, L], FP16, isOutput=False)
    xk = nc.declare_dram_parameter("xk", [C, L], FP16, isOutput=False)
    xv = nc.declare_dram_parameter("xv", [C, L], FP16, isOutput=False)
    wqT = nc.declare_dram_parameter("wqT", [C, C], FP16, isOutput=False)
    wkT = nc.declare_dram_parameter("wkT", [C, C], FP16, isOutput=False)
    wv = nc.declare_dram_parameter("wv", [C, C], FP16, isOutput=False)
    woT = nc.declare_dram_parameter("woT", [C, C], FP16, isOutput=False)
    rows = nc.declare_dram_parameter("rows", [4, C], FP16, isOutput=False)
    bvo_pm = nc.declare_dram_parameter("bvo_pm", [P, 2 * CC], F32, isOutput=False)
    out = nc.declare_dram_parameter("out", [C, L], FP16, isOutput=True)

    xv_v = xv.rearrange("(cc p) l -> p cc l", p=P)
    out_v = out.rearrange("(m p) l -> p m l", p=P)
    wq_view = wqT.rearrange("(cc p) o -> p cc o", p=P)
    wk_view = wkT.rearrange("(cc p) o -> p cc o", p=P)
    wv_view = wv.rearrange("(ee p) c -> p ee c", p=P)
    wo_view = woT.rearrange("(cc p) o -> p cc o", p=P)

    with tile.TileContext(nc) as tc:
        with (
            tc.tile_pool(name="const", bufs=1) as const,
            tc.tile_pool(name="psum", bufs=1, space="PSUM") as psum,
        ):
            bvo_sb = const.tile([P, 2 * CC], F32)
            b3_sb = const.tile([P, CC], F32)
            bv16_sb = const.tile([P, CC], FP16)
            rows_sb = const.tile([1, 4, C], FP16)
            wq_sb = const.tile([P, CC, C], FP16)
            wk_sb = const.tile([P, CC, C], FP16)
            wv_sb = const.tile([P, CC, C], FP16)
            wo_sb = const.tile([P, CC, C], FP16)
            H_sb = const.tile([P, CC, C], FP16)  # H = Xk Xq^T chunks
            U_sb = const.tile([P, CC, C], FP16)  # U = H^T WkT chunks
            w2_sb = const.tile([P, NPAIR, C], FP16)  # W2T = blockdiag(An) @ woT
            w3_sb = const.tile([P, CC, C], FP16)  # W3T = WvT @ W2T
            r_all = const.tile([P, NPAIR], F32)
            attn_all = const.tile([P, NPAIR, P], F32)
            attn_n = const.tile([P, NPAIR, P], FP16)
            z_all = const.tile([P, NPAIR], F32)
            # rotated working buffers
            xq_b = const.tile([P, XTB, NSUB, C], FP16)
            xk_b = const.tile([P, XTB, NSUB, C], FP16)
            xv_b = const.tile([P, XVB, CC, LCHUNK], FP16)
            o_b = const.tile([P, OTB, CC, LCHUNK], FP16)

            # PSUM bank map (8 banks of 512 f32):
            psum_G = psum.tile([P, CC, 512], F32)  # banks 0-3: G/U, then W3
            psum_S = psum.tile([P, 512], F32)  # bank 4: S; later 3rd ps_o
            ps_wb = psum.tile([P, 512], F32)  # bank 5: W2 fold, then b3
            ps_oA = psum.tile([P, 512], F32)  # bank 6
            ps_oB = psum.tile([P, 512], F32)  # bank 7
            ps_o_rot = [ps_oA, ps_oB, psum_S]

            # off-diagonal 64-blocks of the pair-packed attention are zero
            # for the whole program; exp only ever rewrites the diagonals
            nc.vector.memset(attn_all[:], 0.0)

            def one_pass(it):
                # ------------- phase A: H = Xk Xq^T (token-major) --------
                for lc in range(NLC):
                    ls = lc * LCHUNK
                    bi = (it * NLC + lc) % XTB
                    # one XBAR transpose per chunk: [C, 512] ->
                    # [128 tok, 4 sub, C] token-major
                    nc.sync.dma_start_transpose(
                        xq_b[:, bi, :, :], xq[:, ls : ls + LCHUNK]
                    )
                    nc.sync.dma_start_transpose(
                        xk_b[:, bi, :, :], xk[:, ls : ls + LCHUNK]
                    )
                    if lc == 1:
                        nc.sync.dma_start(wk_sb[:], wk_view[:])
                    elif lc == 2:
                        nc.sync.dma_start(wq_sb[:], wq_view[:])
                    elif lc == NLC - 1:
                        # fold-stage weights ride behind the last transposes
                        nc.sync.dma_start(wv_sb[:], wv_view[:])
                        nc.sync.dma_start(wo_sb[:], wo_view[:])
                        nc.sync.dma_start(rows_sb[:], rows[:])
                        nc.sync.dma_start(bvo_sb[:], bvo_pm[:])
                    for s in range(NSUB):
                        for cc in range(CC):
                            nc.tensor.matmul(
                                psum_G[:, cc, :],
                                xk_b[:, bi, s, cc * P : (cc + 1) * P],
                                xq_b[:, bi, s, :],
                                start=(lc == 0 and s == 0),
                                stop=(lc == NLC - 1 and s == NSUB - 1),
                            )

                # xv prefetch rides the ACT queue; lands during the U/S/fold
                # tail while the DMA engines are otherwise idle
                def prefetch_xv(lc):
                    if lc < NLC:
                        bi = (it * NLC + lc) % XVB
                        nc.scalar.dma_start(
                            xv_b[:, bi, :, :],
                            xv_v[:, :, lc * LCHUNK : (lc + 1) * LCHUNK],
                        )

                prefetch_xv(0)
                prefetch_xv(1)

                # H chunks PSUM -> SBUF fp16 (DVE/ACT split halves latency)
                for cc in range(CC):
                    if cc % 2 == 0:
                        nc.vector.tensor_copy(H_sb[:, cc, :], psum_G[:, cc, :])
                    else:
                        nc.scalar.activation(
                            H_sb[:, cc, :], psum_G[:, cc, :], AF.Copy
                        )
                # U[c,:] = sum_e H[e,c-slice]^T WkT[e,:] — same banks (WAR
                # on the H copies, range-tracked)
                for cc in range(CC):
                    for ee in range(CC):
                        nc.tensor.matmul(
                            psum_G[:, cc, :],
                            H_sb[:, ee, cc * P : (cc + 1) * P],
                            wk_sb[:, ee, :],
                            start=(ee == 0),
                            stop=(ee == CC - 1),
                        )
                for cc in range(CC):
                    if cc % 2 == 0:
                        nc.vector.tensor_copy(U_sb[:, cc, :], psum_G[:, cc, :])
                    else:
                        nc.scalar.activation(
                            U_sb[:, cc, :], psum_G[:, cc, :], AF.Copy
                        )
                # S (block-diagonal by head pair) + rank-1 bias terms
                for pp in range(NPAIR):
                    sl = slice(pp * P, (pp + 1) * P)
                    for cc in range(CC):
                        nc.tensor.matmul(
                            psum_S[:, sl],
                            wq_sb[:, cc, sl],
                            U_sb[:, cc, sl],
                            start=(pp == 0 and cc == 0),
                            stop=False,
                        )
                    nc.tensor.matmul(
                        psum_S[:, sl],
                        rows_sb[0:1, R_AQ, sl],
                        rows_sb[0:1, R_BK, sl],
                        start=False,
                        stop=False,
                    )
                    nc.tensor.matmul(
                        psum_S[:, sl],
                        rows_sb[0:1, R_BQ, sl],
                        rows_sb[0:1, R_CK, sl],
                        start=False,
                        stop=(pp == NPAIR - 1),
                    )

                # ---------------- softmax + folds ----------------
                # per-pair chains (exp fused with its rowsum via accum_out)
                # so the W2 fold of pair 0 starts while pair 1 still exps
                S_v = psum_S[:].rearrange("p (pp q) -> p pp q", q=P)
                nc.vector.tensor_copy(bv16_sb[:], bvo_sb[:, 0:CC])
                for pp in range(NPAIR):
                    nc.scalar.activation(
                        attn_all[0:D, pp, 0:D],
                        S_v[0:D, pp, 0:D],
                        AF.Exp,
                        0.0,
                        SCALE,
                        accum_out=z_all[0:D, pp : pp + 1],
                    )
                    nc.scalar.activation(
                        attn_all[D:P, pp, D:P],
                        S_v[D:P, pp, D:P],
                        AF.Exp,
                        0.0,
                        SCALE,
                        accum_out=z_all[D:P, pp : pp + 1],
                    )
                    nc.vector.reciprocal(
                        r_all[:, pp : pp + 1], z_all[:, pp : pp + 1]
                    )
                    nc.vector.tensor_scalar_mul(
                        attn_n[:, pp, :], attn_all[:, pp, :], r_all[:, pp : pp + 1]
                    )
                    nc.tensor.matmul(
                        ps_wb[:],
                        attn_n[:, pp, :],
                        wo_sb[:, pp, :],
                        start=True,
                        stop=True,
                    )
                    nc.vector.tensor_copy(w2_sb[:, pp, :], ps_wb[:])
                # W3T[c,:] = sum_e Wv[e,c] * W2T[e,:]  (into the G banks)
                for cc in range(CC):
                    for ee in range(CC):
                        nc.tensor.matmul(
                            psum_G[:, cc, :],
                            wv_sb[:, ee, cc * P : (cc + 1) * P],
                            w2_sb[:, ee, :],
                            start=(ee == 0),
                            stop=(ee == CC - 1),
                        )
                for cc in range(CC):
                    if cc % 2 == 0:
                        nc.vector.tensor_copy(w3_sb[:, cc, :], psum_G[:, cc, :])
                    else:
                        nc.scalar.activation(
                            w3_sb[:, cc, :], psum_G[:, cc, :], AF.Copy
                        )
                # b3[o] = sum_e W2T[e,o] bv[e] + bo[o], o partition-major
                ps_b3 = ps_wb[:, 0:CC]
                for m in range(CC):
                    for pp in range(NPAIR):
                        nc.tensor.matmul(
                            ps_b3[:, m : m + 1],
                            w2_sb[:, pp, m * P : (m + 1) * P],
                            bv16_sb[:, pp : pp + 1],
                            start=(m == 0 and pp == 0),
                            stop=(m == CC - 1 and pp == NPAIR - 1),
                        )
                nc.vector.tensor_add(b3_sb[:], ps_b3, bvo_sb[:, CC : 2 * CC])

                # ------------- phase B: out = W3 @ xv + b3 ---------------
                for lc in range(NLC):
                    ls = lc * LCHUNK
                    prefetch_xv(lc + 2)
                    xv_t = xv_b[:, (it * NLC + lc) % XVB]
                    o_t4 = o_b[:, lc % OTB]
                    for m in range(CC):
                        ps_o = ps_o_rot[(lc * CC + m) % 3]
                        for cc in range(CC):
                            nc.tensor.matmul(
                                ps_o[:],
                                w3_sb[:, cc, m * P : (m + 1) * P],
                                xv_t[:, cc, :],
                                start=(cc == 0),
                                stop=(cc == CC - 1),
                            )
                        nc.vector.tensor_scalar_add(
                            o_t4[:, m, :], ps_o[:], b3_sb[:, m : m + 1]
                        )
                    nc.scalar.dma_start(out_v[:, :, ls : ls + LCHUNK], o_t4[:])

            for it in range(n_iter):
                one_pass(it)

    nc.compile()
    return nc


_NC_CACHE = None


def _get_nc():
    global _NC_CACHE
    if _NC_CACHE is None:
        _NC_CACHE = build_nc()
    return _NC_CACHE


def _prep_in_maps(query, key, value, wq, bq, wk, bk, wv, bv, wo, bo):
    f16 = np.float16

    def f32(a):
        return np.ascontiguousarray(np.asarray(a, dtype=np.float32))

    wq = np.asarray(wq, np.float32)
    wk = np.asarray(wk, np.float32)
    bq32 = np.asarray(bq, np.float32)
    bk32 = np.asarray(bk, np.float32)
    query = np.asarray(query, np.float32).reshape(B, C, L)
    key = np.asarray(key, np.float32).reshape(B, C, L)
    q16 = np.ascontiguousarray(query.astype(f16))
    k16 = np.ascontiguousarray(key.astype(f16))
    v16 = np.ascontiguousarray(np.asarray(value, np.float32).astype(f16).reshape(B, C, L))
    # host-side rank-1 bias rows: aq = Wq (Xq 1) + L bq, ck = Wk (Xk 1)
    # (use the fp16-rounded inputs so corrections match the device data)
    sq = q16.astype(np.float32).sum(axis=2)  # [B, C]
    sk = k16.astype(np.float32).sum(axis=2)
    aq = sq @ wq.T + L * bq32[None, :]  # [B, C]
    ck = sk @ wk.T  # [B, C]
    bvo = np.concatenate(
        [
            np.asarray(bv, np.float32).reshape(CC, P).T,
            np.asarray(bo, np.float32).reshape(CC, P).T,
        ],
        axis=1,
    )
    shared = {
        "wqT": np.ascontiguousarray(wq.T.astype(f16)),
        "wkT": np.ascontiguousarray(wk.T.astype(f16)),
        "wv": np.ascontiguousarray(np.asarray(wv, np.float32).astype(f16)),
        "woT": np.ascontiguousarray(np.asarray(wo, np.float32).T.astype(f16)),
        "bvo_pm": f32(bvo),
    }
    bq16 = bq32.astype(f16)
    in_maps = []
    for b in range(B):
        rows_b = np.stack(
            [aq[b].astype(f16), bq16, bk32.astype(f16), ck[b].astype(f16)], axis=0
        )
        in_maps.append(
            {
                "xq": q16[b],
                "xk": k16[b],
                "xv": v16[b],
                "rows": np.ascontiguousarray(rows_b),
                **shared,
            }
        )
    return in_maps


def kernel(query, key, value, wq, bq, wk, bk, wv, bv, wo, bo):
    nc = _get_nc()
    in_maps = _prep_in_maps(query, key, value, wq, bq, wk, bk, wv, bv, wo, bo)
    res = run_bass_kernel_spmd(nc, in_maps, core_ids=list(range(B)))
    out = np.stack([res.results[b]["out"] for b in range(B)], axis=0)
    return out.reshape(B, C, 64, 64).astype(np.float32)


if __name__ == "__main__":
    rng = np.random.default_rng(0)
    sh = dict(
        query=rng.standard_normal((B, C, 64, 64), dtype=np.float32),
        key=rng.standard_normal((B, C, 64, 64), dtype=np.float32),
        value=rng.standard_normal((B, C, 64, 64), dtype=np.float32),
        wq=rng.standard_normal((C, C), dtype=np.float32) / np.sqrt(C),
        bq=rng.standard_normal((C,), dtype=np.float32) / np.sqrt(C),
        wk=rng.standard_normal((C, C), dtype=np.float32) / np.sqrt(C),
        bk=rng.standard_normal((C,), dtype=np.float32) / np.sqrt(C),
        wv=rng.standard_normal((C, C), dtype=np.float32) / np.sqrt(C),
        bv=rng.standard_normal((C,), dtype=np.float32) / np.sqrt(C),
        wo=rng.standard_normal((C, C), dtype=np.float32) / np.sqrt(C),
        bo=rng.standard_normal((C,), dtype=np.float32) / np.sqrt(C),
    )
    o = kernel(**sh)
    print("kernel output:", o.shape, o.dtype, float(np.abs(o).max()))
